# revision 9
# baseline (speedup 1.0000x reference)
"""BERT encoder (12 layers, B=8, S=512, H=768, NH=12, FF=3072) on 8 TRN2
NeuronCores. Data-parallel over batch: each core runs the full 12-layer
encoder on one batch element; no collectives.

On-chip strategy (per core, per layer):
- Activations live feature-major in SBUF: X^T [H=768 (6 x 128-partition
  chunks), S=512 free]. All projection matmuls then use the DRAM-natural
  weight layout as the stationary operand (lhsT = W[128k, 128m] chunk) and
  X^T chunks as the moving operand; no transposes anywhere on chip.
- Matmul dtype is float32r (fp32 with 12-bit mantissa, full PE rate).
  Weights are pre-rounded to f32r on the host; activations are rounded by
  the PSUM-eviction copy (DVE/ACT write to f32r tiles). End-to-end rel
  error vs the fp32 reference ~2.6e-4 (validated by simulation+probes).
- Attention: scores^T[k,q] per head via K=64 f32r matmuls on 64-partition
  slices; exp (with mask bias + 1/sqrt(64) scale) fused in one ACT op into
  f32r probs; ctx via lhsT = V_tok2 slice with an appended ones-column so
  the softmax denominator lands in the same PSUM tile; per-q normalization
  via DVE reciprocal + a K=1 outer-product broadcast matmul.
- LayerNorm (feature-major = partition direction) via ones-column matmuls
  for sum / sum-of-squares and K=1 outer-product broadcasts of mean/rstd.
- FFN interleaved per 128-wide inter chunk: 6 FFN1 matmuls -> fused
  bias+gelu (exact erf gelu on ACT) -> 6 FFN2 accumulating matmuls, so the
  6 MB intermediate never materializes (6 PSUM banks accumulate the output).
"""
import sys
import numpy as np

sys.path.insert(0, '/opt/trn_rl_repo')

L, B, S, H, NH, DH, FF = 12, 8, 512, 768, 12, 64, 3072
HC = H // 128      # 6 hidden chunks
FC = FF // 128     # 24 ff chunks
SC = S // 128      # 4 token chunks
EPS = 1e-12

_CACHE = {}


def _round_f32r(x):
    b = np.ascontiguousarray(x, np.float32).view(np.uint32)
    r = ((b.astype(np.uint64) + 0x800) & 0xFFFFF000).astype(np.uint32)
    return r.view(np.float32)


def _layernorm(nc, mybir, pf, pr, pst, ps, x_master, vec_t, bias_j, g_j, b_j,
               onec, ones, eps_t, tag, out_master=None, need_f32r=True):
    """LayerNorm over the partition (feature) direction of x_master
    [128, HC*512] fp32. The pre-LN bias (vslot(bias_j,c), per-feature) is
    folded into the f32r stats copy AND into the normalize step.
    Returns (master_out, f32r_out_or_None)."""
    F32 = mybir.dt.float32
    F32R = mybir.dt.float32r
    ALU = mybir.AluOpType
    AFT = mybir.ActivationFunctionType

    def sl(t, c):
        return t[:, c * 512:(c + 1) * 512]

    def vslot(j, c):
        return vec_t[:, j * HC + c: j * HC + c + 1]

    x_r = pr.tile([128, HC * 512], F32R, tag="actr", name=f"lnxr_{tag}")
    for c in range(HC):
        nc.vector.tensor_scalar(sl(x_r, c), sl(x_master, c),
                                vslot(bias_j, c), None, ALU.add)
    sq_r = pr.tile([128, HC * 512], F32R, tag="actr", name=f"lnsq_{tag}")
    nc.vector.tensor_tensor(sq_r[:], x_r[:], x_r[:], ALU.mult)

    sum_ps = ps.tile([1, 512], F32, tag="ps", name=f"lnsum_{tag}")
    for c in range(HC):
        nc.tensor.matmul(sum_ps[:], onec[:, :], sl(x_r, c), start=(c == 0),
                         stop=(c == HC - 1))
    sq_ps = ps.tile([1, 512], F32, tag="ps", name=f"lnsqs_{tag}")
    for c in range(HC):
        nc.tensor.matmul(sq_ps[:], onec[:, :], sl(sq_r, c), start=(c == 0),
                         stop=(c == HC - 1))

    mean32 = pst.tile([1, 512], F32, tag="stat32", bufs=4, name=f"mean_{tag}")
    nc.vector.tensor_scalar_mul(mean32[:], sum_ps[:], 1.0 / H)
    msq = pst.tile([1, 512], F32, tag="stat32", bufs=4, name=f"msq_{tag}")
    nc.vector.tensor_scalar_mul(msq[:], sq_ps[:], 1.0 / H)
    var = pst.tile([1, 512], F32, tag="stat32", bufs=4, name=f"var_{tag}")
    nc.vector.tensor_tensor(var[:], mean32[:], mean32[:], ALU.mult)
    nc.vector.tensor_tensor(var[:], msq[:], var[:], ALU.subtract)
    sd = pst.tile([1, 512], F32, tag="stat32", bufs=4, name=f"sd_{tag}")
    nc.scalar.activation(sd[:], var[:], AFT.Sqrt, bias=eps_t[0:1, :])
    rstd = pst.tile([1, 512], F32, tag="stat32", bufs=4, name=f"rstd_{tag}")
    nc.vector.reciprocal(rstd[:], sd[:])
    mean_r = pst.tile([1, 512], F32R, tag="statr", name=f"meanr_{tag}")
    nc.vector.tensor_copy(mean_r[:], mean32[:])
    rstd_r = pst.tile([1, 512], F32R, tag="statr", name=f"rstdr_{tag}")
    nc.vector.tensor_copy(rstd_r[:], rstd[:])

    mb_ps = ps.tile([128, 512], F32, tag="ps", name=f"mb_{tag}")
    nc.tensor.matmul(mb_ps[:], ones[0:1, :], mean_r[:], start=True, stop=True)
    rb_ps = ps.tile([128, 512], F32, tag="ps", name=f"rb_{tag}")
    nc.tensor.matmul(rb_ps[:], ones[0:1, :], rstd_r[:], start=True, stop=True)
    rb_sb = pst.tile([128, 512], F32, tag="rb", bufs=2, name=f"rbsb_{tag}")
    nc.vector.tensor_copy(rb_sb[:], rb_ps[:])

    out = out_master if out_master is not None else pf.tile(
        [128, HC * 512], F32, tag="actf", name=f"lnout_{tag}")
    out_r = None
    if need_f32r:
        out_r = pr.tile([128, HC * 512], F32R, tag="actr", name=f"lnoutr_{tag}")
    for c in range(HC):
        t1 = pst.tile([128, 512], F32, tag="lntmp", bufs=1,
                      name=f"lntmp_{tag}_{c}")
        nc.vector.tensor_tensor(t1[:], sl(x_master, c), mb_ps[:], ALU.subtract)
        nc.vector.tensor_scalar(t1[:], t1[:], vslot(bias_j, c), None, ALU.add)
        nc.vector.tensor_tensor(t1[:], t1[:], rb_sb[:], ALU.mult)
        nc.vector.tensor_scalar(sl(out, c), t1[:], vslot(g_j, c),
                                vslot(b_j, c), ALU.mult, ALU.add)
        if need_f32r:
            nc.vector.tensor_copy(sl(out_r, c), sl(out, c))
    return out, out_r


def _build_program():
    import concourse.tile as tile
    from concourse import bacc, mybir

    F32 = mybir.dt.float32
    F32R = mybir.dt.float32r
    AFT = mybir.ActivationFunctionType
    ALU = mybir.AluOpType

    nc = bacc.Bacc("TRN2", target_bir_lowering=False, debug=False)

    hsT = nc.dram_tensor("hsT", [H, S], F32, kind="ExternalInput").ap()
    maskT = nc.dram_tensor("maskT", [128, SC], F32, kind="ExternalInput").ap()
    Wq = nc.dram_tensor("Wq", [L, H, H], F32R, kind="ExternalInput").ap()
    Wk = nc.dram_tensor("Wk", [L, H, H], F32R, kind="ExternalInput").ap()
    Wv = nc.dram_tensor("Wv", [L, H, H], F32R, kind="ExternalInput").ap()
    Wo = nc.dram_tensor("Wo", [L, H, H], F32R, kind="ExternalInput").ap()
    Wi = nc.dram_tensor("Wi", [L, H, FF], F32R, kind="ExternalInput").ap()
    Wo2 = nc.dram_tensor("Wo2", [L, FF, H], F32R, kind="ExternalInput").ap()
    # packed per-layer 768-vecs: bq,bk,bo_eff,g1,b1,g2,b2,bo2 -> [L,128,8*HC]
    vecs = nc.dram_tensor("vecs", [L, 128, 8 * HC], F32,
                          kind="ExternalInput").ap()
    biv = nc.dram_tensor("biv", [L, 128, FC], F32, kind="ExternalInput").ap()
    outT = nc.dram_tensor("outT", [H, S], F32, kind="ExternalOutput").ap()

    with tile.TileContext(nc) as tc, \
            nc.allow_low_precision(reason="f32r matmul pipeline"):
        with (
            tc.tile_pool(name="persist", bufs=1) as pp,
            tc.tile_pool(name="actf", bufs=3) as pf,     # fp32 [128,3072]
            tc.tile_pool(name="actr", bufs=2) as pr,     # f32r [128,3072]
            tc.tile_pool(name="w768", bufs=9) as pw,    # weight chunks
            tc.tile_pool(name="small", bufs=2) as psm,
            tc.tile_pool(name="probs", bufs=2) as ppr,
            tc.tile_pool(name="inter", bufs=2) as pit,
            tc.tile_pool(name="bias", bufs=2) as pb,
            tc.tile_pool(name="stat", bufs=2) as pst,
            tc.tile_pool(name="psum", bufs=2, space="PSUM") as ps,
        ):
            ones32 = pp.tile([128, 128], F32, tag="ones32", name="ones32")
            nc.vector.memset(ones32[:], 1.0)
            zeros32 = pp.tile([128, 64], F32, tag="zeros32", name="zeros32")
            nc.vector.memset(zeros32[:], 0.0)
            ones = pp.tile([128, 128], F32R, tag="ones", name="ones")
            nc.vector.tensor_copy(ones[:], ones32[:])
            onec = pp.tile([128, 1], F32R, tag="onec", name="onec")
            nc.vector.tensor_copy(onec[:], ones32[:, 0:1])
            mask_t = pp.tile([128, SC], F32, tag="mask", name="mask_t")
            nc.sync.dma_start(mask_t[:], maskT)
            eps_t = pp.tile([1, 1], F32, tag="eps", name="eps_t")
            nc.vector.memset(eps_t[:], EPS)

            xT = pp.tile([128, HC * 512], F32, tag="xT", name="xT")
            nc.sync.dma_start(xT[:].rearrange("p (c s) -> p c s", c=HC),
                              hsT.rearrange("(c p) s -> p c s", p=128))

            qT = pp.tile([128, HC * 512], F32R, tag="qT", name="qT")
            kT = pp.tile([128, HC * 512], F32R, tag="kT", name="kT")
            ctxT = pp.tile([128, HC * 512], F32R, tag="ctxT", name="ctxT")
            # v_tok2: [s-chunk][head][128 cols]; even head [v(64)|1|z63],
            # odd head [1|z63|v(64)]
            vt = pp.tile([128, SC * NH * 128], F32R, tag="vt", name="vt")
            vt4 = vt[:].rearrange("p (sc h c) -> p sc h c", sc=SC, h=NH)
            nc.vector.tensor_copy(
                vt4[:, :, 0::2, 64:65],
                ones32[:, None, None, 0:1].broadcast_to([128, SC, 6, 1]))
            nc.vector.tensor_copy(
                vt4[:, :, 0::2, 65:128],
                zeros32[:, None, None, 0:63].broadcast_to([128, SC, 6, 63]))
            nc.vector.tensor_copy(
                vt4[:, :, 1::2, 0:1],
                ones32[:, None, None, 0:1].broadcast_to([128, SC, 6, 1]))
            nc.vector.tensor_copy(
                vt4[:, :, 1::2, 1:64],
                zeros32[:, None, None, 0:63].broadcast_to([128, SC, 6, 63]))

            def mmslice(t, c):
                return t[:, c * 512:(c + 1) * 512]

            for li in range(L):
                vec_t = pb.tile([128, 8 * HC], F32, tag="vec",
                                name=f"vec_{li}")
                nc.sync.dma_start(vec_t[:], vecs[li])
                bi_t = pb.tile([128, FC], F32, tag="biv", name=f"biv_{li}")
                nc.sync.dma_start(bi_t[:], biv[li])

                def vslot(j, c):
                    return vec_t[:, j * HC + c: j * HC + c + 1]

                x_r = pr.tile([128, HC * 512], F32R, tag="actr",
                              name=f"xr_{li}")
                nc.vector.tensor_copy(x_r[:], xT[:])

                wq_t = [pw.tile([128, H], F32R, tag="w768",
                                name=f"wq_{li}_{c}") for c in range(HC)]
                for c in range(HC):
                    nc.sync.dma_start(wq_t[c][:], Wq[li, c * 128:(c + 1) * 128, :])
                wk_t = [pw.tile([128, H], F32R, tag="w768",
                                name=f"wk_{li}_{c}") for c in range(HC)]
                for c in range(HC):
                    nc.sync.dma_start(wk_t[c][:], Wk[li, c * 128:(c + 1) * 128, :])
                wv_t = [pw.tile([128, H], F32R, tag="w768",
                                name=f"wv_{li}_{c}") for c in range(HC)]
                for c in range(HC):
                    nc.sync.dma_start(wv_t[c][:], Wv[li, c * 128:(c + 1) * 128, :])

                for m in range(HC):
                    q_ps = ps.tile([128, 512], F32, tag="ps",
                                   name=f"qps_{li}_{m}")
                    for c in range(HC):
                        nc.tensor.matmul(q_ps[:],
                                         wq_t[c][:, m * 128:(m + 1) * 128],
                                         mmslice(x_r, c), start=(c == 0),
                                         stop=(c == HC - 1))
                    nc.vector.tensor_scalar(mmslice(qT, m), q_ps[:],
                                            vslot(0, m), None, ALU.add)
                for m in range(HC):
                    k_ps = ps.tile([128, 512], F32, tag="ps",
                                   name=f"kps_{li}_{m}")
                    for c in range(HC):
                        nc.tensor.matmul(k_ps[:],
                                         wk_t[c][:, m * 128:(m + 1) * 128],
                                         mmslice(x_r, c), start=(c == 0),
                                         stop=(c == HC - 1))
                    nc.vector.tensor_scalar(mmslice(kT, m), k_ps[:],
                                            vslot(1, m), None, ALU.add)

                # V projection, token-major into vt (bv folded into bo_eff)
                for sc in range(SC):
                    for half in range(2):
                        v_ps = ps.tile([128, 384], F32, tag="ps",
                                       name=f"vps_{li}_{sc}_{half}")
                        for c in range(HC):
                            nc.tensor.matmul(
                                v_ps[:],
                                x_r[:, c * 512 + sc * 128:
                                    c * 512 + (sc + 1) * 128],
                                wv_t[c][:, half * 384:(half + 1) * 384],
                                start=(c == 0), stop=(c == HC - 1))
                        v3 = v_ps[:].rearrange("p (h x c) -> p h x c",
                                               h=3, x=2)
                        nc.vector.tensor_copy(
                            vt4[:, sc, half * 6 + 0:half * 6 + 6:2, 0:64],
                            v3[:, :, 0, :])
                        nc.vector.tensor_copy(
                            vt4[:, sc, half * 6 + 1:half * 6 + 6:2, 64:128],
                            v3[:, :, 1, :])

                wo_t = [pw.tile([128, H], F32R, tag="w768",
                                name=f"wo_{li}_{c}") for c in range(HC)]
                for c in range(HC):
                    nc.sync.dma_start(wo_t[c][:], Wo[li, c * 128:(c + 1) * 128, :])

                # ---- attention per head ----
                for h in range(NH):
                    c, r = h // 2, h % 2
                    o = r * 64
                    probs = ppr.tile([128, SC * 512], F32R, tag="probs",
                                     name=f"probs_{li}_{h}")
                    for kc in range(SC):
                        s_ps = ps.tile([128, 512], F32, tag="ps",
                                       name=f"sps_{li}_{h}_{kc}")
                        nc.tensor.matmul(
                            s_ps[:],
                            kT[o:o + 64,
                               c * 512 + kc * 128: c * 512 + (kc + 1) * 128],
                            qT[o:o + 64, c * 512:(c + 1) * 512],
                            start=True, stop=True)
                        nc.scalar.activation(
                            probs[:, kc * 512:(kc + 1) * 512], s_ps[:],
                            AFT.Exp, bias=mask_t[:, kc:kc + 1],
                            scale=float(1.0 / np.sqrt(DH)))
                    ctx_ps = ps.tile([128, 512], F32, tag="ps",
                                     name=f"cps_{li}_{h}")
                    for kc in range(SC):
                        lhs = (vt4[:, kc, h, 0:65] if r == 0
                               else vt4[:, kc, h, 0:128])
                        nc.tensor.matmul(ctx_ps[0:(65 if r == 0 else 128), :],
                                         lhs, probs[:, kc * 512:(kc + 1) * 512],
                                         start=(kc == 0), stop=(kc == SC - 1))
                    drow = 64 if r == 0 else 0
                    rec = psm.tile([128, 512], F32R, tag="rec",
                                   name=f"rec_{li}_{h}")
                    nc.vector.reciprocal(rec[drow:drow + 1, :],
                                         ctx_ps[drow:drow + 1, :])
                    b_ps = ps.tile([128, 512], F32, tag="ps",
                                   name=f"bps_{li}_{h}")
                    nc.tensor.matmul(b_ps[:], ones[drow:drow + 1, :],
                                     rec[drow:drow + 1, :],
                                     start=True, stop=True)
                    bsb = psm.tile([128, 512], F32, tag="bsb",
                                   name=f"bsb_{li}_{h}")
                    nc.vector.tensor_copy(bsb[:], b_ps[:])
                    nc.vector.tensor_tensor(
                        ctxT[o:o + 64, c * 512:(c + 1) * 512],
                        ctx_ps[o:o + 64, :], bsb[o:o + 64, :], ALU.mult)

                # ---- attn output projection + residual ----
                ap_ = pf.tile([128, HC * 512], F32, tag="actf",
                              name=f"ap_{li}")
                for m in range(HC):
                    a_ps = ps.tile([128, 512], F32, tag="ps",
                                   name=f"aps_{li}_{m}")
                    for c in range(HC):
                        nc.tensor.matmul(a_ps[:],
                                         wo_t[c][:, m * 128:(m + 1) * 128],
                                         mmslice(ctxT, c), start=(c == 0),
                                         stop=(c == HC - 1))
                    nc.vector.tensor_tensor(mmslice(ap_, m), a_ps[:],
                                            mmslice(xT, m), ALU.add)

                lo, lo_r = _layernorm(nc, mybir, pf, pr, pst, ps, ap_, vec_t,
                                      2, 3, 4, onec, ones, eps_t,
                                      f"l1_{li}")

                # ---- FFN interleaved ----
                acc_ps = [ps.tile([128, 512], F32, tag="ffacc", bufs=6,
                                  name=f"facc_{li}_{m}") for m in range(HC)]
                for g in range(4):
                    wig = [pw.tile([128, H], F32R, tag="w768",
                                   name=f"wi_{li}_{g}_{c}") for c in range(HC)]
                    for c in range(HC):
                        nc.sync.dma_start(
                            wig[c][:],
                            Wi[li, c * 128:(c + 1) * 128, g * 768:(g + 1) * 768])
                    for fg in range(HC):
                        f = g * HC + fg
                        f1_ps = ps.tile([128, 512], F32, tag="ps",
                                        name=f"f1_{li}_{f}")
                        for c in range(HC):
                            nc.tensor.matmul(
                                f1_ps[:],
                                wig[c][:, fg * 128:(fg + 1) * 128],
                                mmslice(lo_r, c), start=(c == 0),
                                stop=(c == HC - 1))
                        inter = pit.tile([128, 512], F32R, tag="inter",
                                         name=f"it_{li}_{f}")
                        nc.scalar.activation(inter[:], f1_ps[:], AFT.Gelu,
                                             bias=bi_t[:, f:f + 1])
                        wo2_t = pw.tile([128, H], F32R, tag="w768",
                                        name=f"wo2_{li}_{f}")
                        nc.sync.dma_start(wo2_t[:],
                                          Wo2[li, f * 128:(f + 1) * 128, :])
                        for m in range(HC):
                            nc.tensor.matmul(
                                acc_ps[m][:], wo2_t[:, m * 128:(m + 1) * 128],
                                inter[:], start=(f == 0), stop=(f == FC - 1),
                                skip_group_check=True)

                fp_ = pf.tile([128, HC * 512], F32, tag="actf",
                              name=f"fp_{li}")
                for m in range(HC):
                    nc.vector.tensor_tensor(mmslice(fp_, m), acc_ps[m][:],
                                            mmslice(lo, m), ALU.add)

                _layernorm(nc, mybir, pf, pr, pst, ps, fp_, vec_t,
                           7, 5, 6, onec, ones, eps_t, f"l2_{li}",
                           out_master=xT, need_f32r=False)

            nc.sync.dma_start(outT.rearrange("(c p) s -> p c s", p=128),
                              xT[:].rearrange("p (c s) -> p c s", c=HC))

    nc.compile()
    return nc


def _get_runner():
    if "runner" in _CACHE:
        return _CACHE["runner"]
    import jax
    from jax.sharding import Mesh, PartitionSpec
    from jax.experimental.shard_map import shard_map
    from concourse import mybir
    from concourse.bass2jax import (_bass_exec_p, install_neuronx_cc_hook,
                                    partition_id_tensor)

    install_neuronx_cc_hook()
    nc = _build_program()

    pname = nc.partition_id_tensor.name if nc.partition_id_tensor else None
    in_names, out_names, out_avals, zero_outs = [], [], [], []
    for alloc in nc.m.functions[0].allocations:
        if not isinstance(alloc, mybir.MemoryLocationSet):
            continue
        name = alloc.memorylocations[0].name
        if alloc.kind == "ExternalInput":
            if name == pname:
                continue
            in_names.append(name)
        elif alloc.kind == "ExternalOutput":
            out_names.append(name)
            shape = tuple(alloc.tensor_shape)
            dtype = mybir.dt.np(alloc.dtype)
            out_avals.append(jax.core.ShapedArray(shape, dtype))
            zero_outs.append(np.zeros(shape, dtype))
    n_params = len(in_names)
    n_outs = len(out_avals)
    all_in_names = list(in_names) + list(out_names)
    if pname is not None:
        all_in_names = all_in_names + [pname]

    def _body(*args):
        operands = list(args)
        if pname is not None:
            operands.append(partition_id_tensor())
        outs = _bass_exec_p.bind(
            *operands,
            out_avals=tuple(out_avals),
            in_names=tuple(all_in_names),
            out_names=tuple(out_names),
            lowering_input_output_aliases=(),
            sim_require_finite=False,
            sim_require_nnan=False,
            nc=nc,
        )
        return tuple(outs)

    devices = jax.devices()[:B]
    mesh = Mesh(np.asarray(devices), ("core",))
    in_specs = (PartitionSpec("core"),) * (n_params + n_outs)
    out_specs = (PartitionSpec("core"),) * n_outs
    donate = tuple(range(n_params, n_params + n_outs))
    jitted = jax.jit(
        shard_map(_body, mesh=mesh, in_specs=in_specs, out_specs=out_specs,
                  check_rep=False),
        donate_argnums=donate, keep_unused=True)

    runner = {
        "jit": jitted, "in_names": in_names, "out_names": out_names,
        "zero_outs": zero_outs, "mesh": mesh, "devices": devices,
    }
    _CACHE["runner"] = runner
    return runner


def _prep_core_inputs(inputs):
    hs = np.asarray(inputs["hidden_states"], np.float32)
    mask = np.asarray(inputs["attention_mask"], np.float32)
    Wq = _round_f32r(inputs["Wq"])
    Wk = _round_f32r(inputs["Wk"])
    Wv = _round_f32r(inputs["Wv"])
    Wo = _round_f32r(inputs["Wo"])
    Wi = _round_f32r(inputs["Wi"])
    Wo2 = _round_f32r(inputs["Wo2"])
    bq = np.asarray(inputs["bq"], np.float32)
    bk = np.asarray(inputs["bk"], np.float32)
    bv = np.asarray(inputs["bv"], np.float32)
    bo = np.asarray(inputs["bo"], np.float32)
    bi = np.asarray(inputs["bi"], np.float32)
    bo2 = np.asarray(inputs["bo2"], np.float32)
    g1 = np.asarray(inputs["ln1_g"], np.float32)
    b1 = np.asarray(inputs["ln1_b"], np.float32)
    g2 = np.asarray(inputs["ln2_g"], np.float32)
    b2 = np.asarray(inputs["ln2_b"], np.float32)

    # fold bv into bo: (ctx + bv) @ Wo + bo == ctx @ Wo + (bo + bv @ Wo)
    bo_eff = (bo.astype(np.float64)
              + np.einsum("lh,lho->lo", bv.astype(np.float64),
                          np.asarray(inputs["Wo"], np.float64))
              ).astype(np.float32)

    def pack768(v):  # [L,768] -> [L,128,HC] with [l,p,c] = v[l, c*128+p]
        return np.ascontiguousarray(v.reshape(L, HC, 128).transpose(0, 2, 1))

    vecs = np.stack([pack768(v) for v in
                     (bq, bk, bo_eff, g1, b1, g2, b2, bo2)], axis=2)
    # [L,128,8,HC] -> [L,128,8*HC]
    vecs = np.ascontiguousarray(vecs.reshape(L, 128, 8 * HC))
    biv = np.ascontiguousarray(bi.reshape(L, FC, 128).transpose(0, 2, 1))

    per_core = {
        "hsT": [np.ascontiguousarray(hs[b].T) for b in range(B)],
        "maskT": [np.ascontiguousarray(mask[b, 0, 0].reshape(SC, 128).T)
                  for b in range(B)],
    }
    for name, arr in (("Wq", Wq), ("Wk", Wk), ("Wv", Wv), ("Wo", Wo),
                      ("Wi", Wi), ("Wo2", Wo2), ("vecs", vecs), ("biv", biv)):
        per_core[name] = [arr] * B
    return per_core


def run_on_device(inputs, n_timing_runs=0):
    """Execute; returns (output [B,S,H] fp32, exec_seconds or None)."""
    import jax
    from jax.sharding import NamedSharding, PartitionSpec
    runner = _get_runner()
    per_core = _prep_core_inputs(inputs)
    devices = runner["devices"]
    mesh = runner["mesh"]
    sharding = NamedSharding(mesh, PartitionSpec("core"))

    global_args = []
    for name in runner["in_names"]:
        shards = per_core[name]
        arrs = [jax.device_put(shards[c], devices[c]) for c in range(B)]
        gshape = (B * shards[0].shape[0],) + shards[0].shape[1:]
        global_args.append(
            jax.make_array_from_single_device_arrays(gshape, sharding, arrs))

    def zeros_args():
        outs = []
        for z in runner["zero_outs"]:
            arrs = [jax.device_put(z, devices[c]) for c in range(B)]
            gshape = (B * z.shape[0],) + z.shape[1:]
            outs.append(jax.make_array_from_single_device_arrays(
                gshape, sharding, arrs))
        return outs

    out_arrs = runner["jit"](*global_args, *zeros_args())
    jax.block_until_ready(out_arrs)

    exec_s = None
    if n_timing_runs > 0:
        import time
        times = []
        for _ in range(n_timing_runs):
            zo = zeros_args()
            jax.block_until_ready(zo)
            t0 = time.perf_counter()
            out_arrs = runner["jit"](*global_args, *zo)
            jax.block_until_ready(out_arrs)
            times.append(time.perf_counter() - t0)
        exec_s = min(times)

    outT = np.asarray(out_arrs[0]).reshape(B, H, S)
    out = np.ascontiguousarray(outT.transpose(0, 2, 1))
    return out, exec_s


def kernel(**inputs) -> np.ndarray:
    out, _ = run_on_device(inputs, n_timing_runs=0)
    return out
